# revision 39
# baseline (speedup 1.0000x reference)
"""MetaGatedTitansLayer Trainium2 kernel (v4: fp16 I/O, transposed space,
cross-stage software pipelining).

Pure data-parallel: batch B=256 sharded 32/core across 8 cores.
Host pre-transposes old_state[b] -> oldT (j, i) and converts to fp16; the
device computes entirely in the transposed layout:
  mc   = old @ q     ->  mc[i]  = sum_j oldT[j,i] q[j]   (PE matvec, fp16)
  pred = old @ k     ->  perr   = pred - v               (PE matvec, fp16)
  newT = oma*oldT + k[j]*(eta*err)[i]                    (DVE ts 4x +
                                                          stt split DVE/Pool)
and streams newT back as fp16; host transposes/casts to fp32.

Pipelining: the emission order interleaves three groups' work so every
in-order engine has independent work queued behind stalled ops:
  merged step g = stage B of group g  +  stage C of g-1  +  stage A of g+1
"""

import sys

import numpy as np

if "/opt/trn_rl_repo" not in sys.path:
    sys.path.insert(0, "/opt/trn_rl_repo")

B, D = 256, 512
NCORES = 8
LB = B // NCORES          # 32 local batch per core
G = 8                     # group size (4 groups of 8)
NG = LB // G
LN_EPS, L2_EPS = 1e-5, 1e-12
TD = 2 * D + 2            # 1026
MAGIC = 0x5F3759DF

_CACHE: dict = {}


def _build():
    import concourse.bass as bass
    import concourse.mybir as mybir
    import concourse.tile as tile
    from concourse import bacc
    from concourse.masks import make_identity

    f32 = mybir.dt.float32
    f16 = mybir.dt.float16
    i32 = mybir.dt.int32
    AF = mybir.ActivationFunctionType
    OP = mybir.AluOpType
    AX = mybir.AxisListType

    nc = bacc.Bacc("TRN2", target_bir_lowering=False, debug=False,
                   num_devices=NCORES)

    # ---------------- DRAM I/O ----------------
    oldT_d = nc.dram_tensor("oldT", [LB, D, D], f16, kind="ExternalInput").ap()
    xs_d = nc.dram_tensor("xs", [LB, D], f32, kind="ExternalInput").ap()
    it_d = nc.dram_tensor("it", [LB, D], f32, kind="ExternalInput").ap()
    wqT_d = nc.dram_tensor("wqT", [D, D], f16, kind="ExternalInput").ap()
    w1T_d = nc.dram_tensor("w1T", [2 * D, D], f16, kind="ExternalInput").ap()
    w2T_d = nc.dram_tensor("w2T", [D, TD], f16, kind="ExternalInput").ap()
    wkvT_d = nc.dram_tensor("wkvT", [D, TD], f16, kind="ExternalInput").ap()
    # cpack = [lng, lnb, b1, b2[:D], b2[D:2D], b2[2D:]+bae] (host-packed)
    cpk_d = nc.dram_tensor("cpack", [5 * D + 2], f16,
                           kind="ExternalInput").ap()
    # npack = [n1g, n1b]
    npk_d = nc.dram_tensor("npack", [2 * D], f16, kind="ExternalInput").ap()
    out_d = nc.dram_tensor("out", [LB, D, D], f16, kind="ExternalOutput").ap()

    def bcast(dst, src_1d):
        # DMA-replicate a 1-D DRAM vector across partitions.
        p = dst.shape[0]
        src = bass.AP(tensor=src_1d.tensor, offset=src_1d.offset,
                      ap=[[0, p]] + list(src_1d.ap))
        nc.gpsimd.dma_start(out=dst, in_=src)

    with tile.TileContext(nc) as tc, bass.ExitStack() as ctx:
        cst = ctx.enter_context(tc.tile_pool(name="cst", bufs=1))
        grp = ctx.enter_context(tc.tile_pool(name="grp", bufs=1))
        dbl = ctx.enter_context(tc.tile_pool(name="dbl", bufs=3))
        ps = ctx.enter_context(tc.tile_pool(name="ps", bufs=1, space="PSUM"))

        # ---------------- constants / weights ----------------
        # phase-1 inputs first so the LNs start early
        npk = cst.tile([LB, 2 * D], f16); bcast(npk, npk_d)
        xs_t = cst.tile([LB, D], f32)
        nc.sync.dma_start(out=xs_t, in_=xs_d)
        it_t = cst.tile([LB, D], f32)
        nc.sync.dma_start(out=it_t, in_=it_d)
        wqT = cst.tile([128, 4, D], f16)
        nc.sync.dma_start(out=wqT, in_=wqT_d.rearrange("(c p) m -> p c m", p=128))

        ident = cst.tile([128, 128], f32)
        make_identity(nc, ident)
        ident16 = cst.tile([128, 128], f16)
        make_identity(nc, ident16)
        negI16 = cst.tile([G, G], f16)
        nc.vector.tensor_scalar(negI16, ident16[0:G, 0:G], -1.0, None,
                                op0=OP.mult)
        ones16 = cst.tile([1, 128], f16)
        nc.vector.memset(ones16, 1.0)

        cpk = cst.tile([G, 5 * D + 2], f16)
        lngb = cpk[:, 0:D]
        lnbb = cpk[:, D:2 * D]
        b1b = cpk[:, 2 * D:3 * D]
        b2gb = cpk[:, 3 * D:4 * D]
        b2bb = cpk[:, 4 * D:5 * D]
        b2aeb = cpk[:, 5 * D:5 * D + 2]

        # ---------------- helpers ----------------
        def rsqrt(y, x, tag, iters=1):
            """y (P,1) f32 <- 1/sqrt(x), DVE only (no ACT table)."""
            p = y.shape[0]
            t = grp.tile([p, 1], f32, tag=f"rt_{tag}", name=f"rt_{tag}")
            yi = y.bitcast(i32)
            nc.vector.tensor_scalar(yi, x.bitcast(i32), 1, None,
                                    op0=OP.logical_shift_right)
            nc.vector.tensor_scalar(yi, yi, -1, MAGIC, op0=OP.mult, op1=OP.add)
            for _ in range(iters):
                nc.vector.tensor_tensor(out=t, in0=x, in1=y, op=OP.mult)
                nc.vector.tensor_tensor(out=t, in0=t, in1=y, op=OP.mult)
                nc.vector.tensor_scalar(t, t, -0.5, 1.5, op0=OP.mult, op1=OP.add)
                nc.vector.tensor_tensor(out=y, in0=y, in1=t, op=OP.mult)

        def layernorm(x, g_bc, b_bc, tag, gb_on_pool=False, iters=1,
                      sums=None):
            """in-place LN over free dim of x (P,512), with bcast gain/bias.
            Row sums / sum-of-squares come from ACT accum_out (cheap on the
            otherwise-idle Act engine); var = E[x^2] - m^2 on tiny DVE ops."""
            p = x.shape[0]
            scr = grp.tile([p, D], f32, tag=f"sc_{tag}", name=f"sc_{tag}")
            ssq = grp.tile([p, 1], f32, tag=f"sq_{tag}", name=f"sq_{tag}")
            mv = grp.tile([p, 2], f32, tag=f"mv_{tag}", name=f"mv_{tag}")
            rs = grp.tile([p, 1], f32, tag=f"rs_{tag}", name=f"rs_{tag}")
            if sums is None:
                sums = grp.tile([p, 1], f32, tag=f"su_{tag}",
                                name=f"su_{tag}")
                nc.scalar.activation(out=scr, in_=x, func=AF.Copy,
                                     accum_out=sums)
            nc.scalar.activation(out=scr, in_=x, func=AF.Square,
                                 accum_out=ssq)
            m = mv[:, 0:1]
            v = mv[:, 1:2]
            nc.vector.tensor_scalar(m, sums, 1.0 / D, None, op0=OP.mult)
            nc.vector.tensor_tensor(out=v, in0=m, in1=m, op=OP.mult)
            nc.vector.tensor_scalar(ssq, ssq, 1.0 / D, None, op0=OP.mult)
            nc.vector.tensor_tensor(out=v, in0=ssq, in1=v, op=OP.subtract)
            rsqrt(rs, v, f"ln_{tag}", iters=iters)
            nc.vector.tensor_scalar(x, x, m, rs,
                                    op0=OP.subtract, op1=OP.mult)
            eng = nc.gpsimd if gb_on_pool else nc.vector
            eng.tensor_tensor(out=x, in0=x, in1=g_bc[:p, :], op=OP.mult)
            eng.tensor_tensor(out=x, in0=x, in1=b_bc[:p, :], op=OP.add)

        def l2row(x, tag, iters=1, out=None):
            """row l2-normalize x (P,512) (eps guard via clamp), optionally
            writing to `out` (e.g. an fp16 tile) in the final scale.
            Sum of squares via ACT Square + accum_out."""
            p = x.shape[0]
            sq = grp.tile([p, D], f32, tag=f"sq2_{tag}", name=f"sq2_{tag}")
            s = grp.tile([p, 1], f32, tag=f"s_{tag}", name=f"s_{tag}")
            ri = grp.tile([p, 1], f32, tag=f"ri_{tag}", name=f"ri_{tag}")
            nc.scalar.activation(out=sq, in_=x, func=AF.Square, accum_out=s)
            rsqrt(ri, s, f"l2_{tag}", iters=iters)
            nc.vector.tensor_scalar(ri, ri, 1.0 / L2_EPS, None, op0=OP.min)
            nc.vector.tensor_scalar(out if out is not None else x, x, ri,
                                    None, op0=OP.mult)

        def pe_transpose_block(psum_out, sb_in, idn):
            k = sb_in.shape[0]
            nc.tensor.transpose(psum_out, sb_in, idn[0:k, 0:k])

        # ---------------- phase 1: batch-level ----------------
        # (stage-A group-0 slab loads are hoisted before phase-1 compute;
        # see emission below)
        tpn_p = ctx.enter_context(tc.tile_pool(name="tpn_p", bufs=3 * G + 2))
        tpns = {0: [None] * G}
        for _bi in range(G):
            _tpn = tpn_p.tile([128, 4, D], f16, tag="tpn", name="tpn")
            nc.sync.dma_start(
                out=_tpn,
                in_=oldT_d[_bi].rearrange("(jc p) i -> p jc i", p=128))
            tpns[0][_bi] = _tpn

        # MLP weights + group consts (first needed in stage B of group 0)
        w1T = cst.tile([128, 8, D], f16)
        nc.sync.dma_start(out=w1T, in_=w1T_d.rearrange("(c p) m -> p c m", p=128))
        bcast(cpk, cpk_d)
        w2T = cst.tile([128, 4, TD], f16)
        nc.sync.dma_start(out=w2T, in_=w2T_d.rearrange("(c p) m -> p c m", p=128))
        wkvT = cst.tile([128, 4, TD], f16)
        nc.sync.dma_start(out=wkvT, in_=wkvT_d.rearrange("(c p) m -> p c m", p=128))

        ph1 = tc.tile_pool(name="ph1", bufs=1)
        p1 = ph1.__enter__()
        n1g32 = npk[:, 0:D]
        n1b32 = npk[:, D:2 * D]
        xsn = xs_t
        layernorm(xsn, n1g32, n1b32, "xsn", iters=2)
        inorm16 = cst.tile([LB, D], f16)

        ing_all = [cst.tile([G, D], f16, name=f"ing{_g}")
                   for _g in range(NG)]

        def emit_inorm_ln():
            inorm = it_t
            layernorm(inorm, n1g32, n1b32, "inorm", iters=2)
            nc.vector.tensor_copy(out=inorm16, in_=inorm)
            for _g in range(NG):
                nc.gpsimd.dma_start(out=ing_all[_g],
                                    in_=inorm16[_g * G:(_g + 1) * G, :])

        # xsnT (128, 4, LB) fp16
        xsnT = cst.tile([128, 4, LB], f16)
        pTx = ps.tile([128, 4, LB], f32, tag="grpP", bufs=1, name="pTx")
        for kc in range(4):
            pe_transpose_block(pTx[:, kc, :], xsn[:, kc * 128:(kc + 1) * 128],
                               ident)
        nc.scalar.copy(out=xsnT, in_=pTx)

        # q = l2norm(xsn @ w_q.T)
        q_rows = p1.tile([LB, D], f32)
        pq = ps.tile([LB, D], f32, tag="grpP", bufs=1, name="pq")
        for kc in range(4):
            nc.tensor.matmul(pq, lhsT=xsnT[:, kc, :], rhs=wqT[:, kc, :],
                             start=(kc == 0), stop=(kc == 3))
        nc.scalar.copy(out=q_rows, in_=pq)
        l2row(q_rows, "q", iters=2)
        qT = cst.tile([128, 4, LB], f16)
        pTq = ps.tile([128, 4, LB], f32, tag="grpP", name="pTq")
        for kc in range(4):
            pe_transpose_block(pTq[:, kc, :],
                               q_rows[:, kc * 128:(kc + 1) * 128], ident)
        nc.scalar.copy(out=qT, in_=pTq)
        ph1.__exit__(None, None, None)

        # ---------------- phase 2: pipelined groups ----------------
        miTps = {}     # g -> miT psum tile
        bss = {}       # g -> stage-B outputs
        mcrows = {}    # (g, bi) -> mcrow tile
        perrs = {}     # (g, bi) -> perr psum
        eerbs = {}     # (g, bi) -> eerb tile

        # ---- stage A pieces ----
        def a_load(g, bi):
            b = g * G + bi
            tpn = tpn_p.tile([128, 4, D], f16, tag="tpn", name="tpn")
            nc.sync.dma_start(
                out=tpn, in_=oldT_d[b].rearrange("(jc p) i -> p jc i", p=128))
            tpns[g][bi] = tpn

        def a_b(g, bi):
            """mc matvec in column form: lhsT = oldT block (j x i), rhs = q
            column -> miT[:, ic, bi] accumulates over jc.  Writes the MLP
            lhsT layout directly (no PSUM row copy, no transposes)."""
            b = g * G + bi
            if tpns[g][bi] is None:
                a_load(g, bi)
            tpn = tpns[g][bi]
            for ic in range(4):
                for jc in range(4):
                    nc.tensor.matmul(miTps[g][:, ic, bi:bi + 1],
                                     lhsT=tpn[:, jc, ic * 128:(ic + 1) * 128],
                                     rhs=qT[:, jc, b:b + 1],
                                     start=(jc == 0), stop=(jc == 3))

        def a_init(g):
            tpns.setdefault(g, [None] * G)
            miTps[g] = ps.tile([128, 4, G], f32, tag="bc", bufs=1,
                               name="miT_ps")

        def a_sched(g):
            """Chunked stage-A emission: loads first, mc matvecs later."""
            return [[("l", bi) for bi in range(G)],
                    [("b", 0), ("b", 1)],
                    [("b", 2), ("b", 3)],
                    [("b", 4), ("b", 5)],
                    [("b", 6), ("b", 7)],
                    []]

        def a_emit(g, chunk):
            for kind, bi in chunk:
                if kind == "l":
                    a_load(g, bi)
                else:
                    a_b(g, bi)

        # ---- stage C pieces ----
        def c1(g, bi):
            bs = bss[g]
            perr = ps.tile([1, D], f32, tag="vec", bufs=3, name="perr")
            for jc in range(4):
                nc.tensor.matmul(perr, lhsT=bs["kT"][:, jc, bi:bi + 1],
                                 rhs=tpns[g][bi][:, jc, :],
                                 start=(jc == 0), stop=False)
            nc.tensor.matmul(perr, lhsT=negI16[:, bi:bi + 1],
                             rhs=bs["vv16"], start=False, stop=True)
            perrs[(g, bi)] = perr

        def c2(g, bi):
            bs = bss[g]
            eerow = dbl.tile([1, D], f16, tag="eerow", bufs=3, name="eerow")
            nc.scalar.mul(eerow, perrs[(g, bi)], bs["etar"][0:1, bi:bi + 1])
            peerb = ps.tile([128, D], f32, tag="bc2", bufs=2, name="peerb")
            nc.tensor.matmul(peerb, lhsT=ones16, rhs=eerow, start=True,
                             stop=True)
            eerb = dbl.tile([128, D], f16, tag="eerb", bufs=3, name="eerb")
            nc.scalar.copy(out=eerb, in_=peerb)
            eerbs[(g, bi)] = eerb

        def c3(g, bi, drain=False):
            bs = bss[g]
            b = g * G + bi
            tpn = tpns[g][bi]
            # Pool supports only plain tensor_tensor (no TensorScalarPtr,
            # no PSUM); jc 2,3 route their rank-1 add through Pool.
            # One whole-slab oma scale (4x DVE; Act in the drain), then the
            # per-jc rank-1 terms.
            if drain and bi % 2 == 1:
                nc.scalar.mul(tpn, tpn, bs["omab"][:, bi:bi + 1])
            else:
                nc.vector.tensor_scalar(tpn, tpn, bs["omab"][:, bi:bi + 1],
                                        None, op0=OP.mult)
            for jc in (2, 3):
                tmp = dbl.tile([128, D], f16, tag="tmp", bufs=3, name="tmp")
                nc.vector.tensor_scalar(tmp, eerbs[(g, bi)],
                                        bs["kTf"][:, jc, bi:bi + 1],
                                        None, op0=OP.mult)
                eng = (nc.vector if drain and (bi + jc) % 2 == 0
                       else nc.gpsimd)
                eng.tensor_tensor(out=tpn[:, jc, :],
                                  in0=tpn[:, jc, :], in1=tmp,
                                  op=OP.add)
            for jc in (0, 1):
                nc.vector.scalar_tensor_tensor(
                    out=tpn[:, jc, :], in0=eerbs[(g, bi)],
                    scalar=bs["kTf"][:, jc, bi:bi + 1],
                    in1=tpn[:, jc, :], op0=OP.mult, op1=OP.add)
            nc.sync.dma_start(
                out=out_d[b].rearrange("(jc p) i -> p jc i", p=128),
                in_=tpn)

        def c_sched(g):
            """Chunked stage-C emission (6 chunks, includes tail)."""
            return [[("c1", 0), ("c1", 1), ("c2", 0)],
                    [("c1", 2), ("c1", 3), ("c2", 1), ("c2", 2),
                     ("c3", 0), ("c3", 1)],
                    [("c1", 4), ("c2", 3), ("c3", 2)],
                    [("c1", 5), ("c2", 4), ("c3", 3)],
                    [("c1", 6), ("c1", 7), ("c2", 5), ("c2", 6),
                     ("c3", 4), ("c3", 5)],
                    [("c2", 7), ("c3", 6), ("c3", 7)]]

        def c_emit(g, chunk, drain=False):
            for kind, bi in chunk:
                if kind == "c3":
                    c3(g, bi, drain=drain)
                else:
                    {"c1": c1, "c2": c2}[kind](g, bi)

        def c_tail(g):
            c2(g, G - 1)
            c3(g, G - 2)
            c3(g, G - 1)

        # ---- stage B ----
        def stage_b(g, fills):
            def fill(i):
                for fn in fills[i]:
                    fn()

            g0 = g * G
            miT = grp.tile([128, 4, G], f16, tag="miT_s", name="miT")
            nc.scalar.copy(out=miT, in_=miTps[g])

            ph = ps.tile([G, D], f32, tag="grpP", bufs=1, name="ph")
            for kc in range(8):
                lhsT = (xsnT[:, kc, g0:g0 + G] if kc < 4
                        else miT[:, kc - 4, :])
                nc.tensor.matmul(ph, lhsT=lhsT, rhs=w1T[:, kc, :],
                                 start=(kc == 0), stop=(kc == 7))
            fill(0)
            hp = grp.tile([G, D], f32, tag="hp", name="hp")
            hsum = grp.tile([G, 1], f32, tag="hsum", name="hsum")
            nc.vector.scalar_tensor_tensor(out=hp, in0=ph, scalar=1.0,
                                           in1=b1b, op0=OP.mult, op1=OP.add,
                                           accum_out=hsum)
            layernorm(hp, lngb, lnbb, "h", gb_on_pool=True, sums=hsum)
            fill(1)
            hp16 = grp.tile([G, D], f16, tag="hp16", name="hp16")
            nc.scalar.activation(out=hp16, in_=hp, func=AF.Relu)

            hT = grp.tile([128, 4, G], f16, tag="hT", name="hT")
            pTh = ps.tile([128, 4, G], f16, tag="grpP", bufs=1, name="pTh")
            for mc2 in range(4):
                pe_transpose_block(pTh[:, mc2, :],
                                   hp16[:, mc2 * 128:(mc2 + 1) * 128],
                                   ident16)
            nc.scalar.copy(out=hT, in_=pTh)

            pg = ps.tile([G, D], f32, tag="grpP", bufs=1, name="pg")
            pbe = ps.tile([G, D], f32, tag="grpP2", bufs=1, name="pbe")
            pae = ps.tile([G, 2], f32, tag="vec", bufs=3, name="pae")
            for mc2 in range(4):
                st, sp = (mc2 == 0), (mc2 == 3)
                nc.tensor.matmul(pg, lhsT=hT[:, mc2, :],
                                 rhs=w2T[:, mc2, 0:D], start=st, stop=sp)
                nc.tensor.matmul(pbe, lhsT=hT[:, mc2, :],
                                 rhs=w2T[:, mc2, D:2 * D], start=st, stop=sp)
                nc.tensor.matmul(pae, lhsT=hT[:, mc2, :],
                                 rhs=w2T[:, mc2, 2 * D:TD], start=st, stop=sp)
            fill(2)

            # gate = 1 + tanh(gamma + b2g); aeb = pae + (b2ae + bae)
            gate = grp.tile([G, D], f32, tag="gate", name="gate")
            nc.vector.tensor_tensor(out=gate, in0=pg, in1=b2gb, op=OP.add)
            gate16 = grp.tile([G, D], f16, tag="gate16", name="gate16")
            nc.scalar.activation(out=gate16, in_=gate, func=AF.Tanh)
            nc.vector.tensor_scalar(gate16, gate16, 1.0, None, op0=OP.add)
            # NOTE: GPSIMD cannot access PSUM (walrus birverifier) — any op
            # reading a p* tile must run on DVE/Act.
            beta16 = grp.tile([G, D], f16, tag="beta16", name="beta16")
            nc.vector.tensor_tensor(out=beta16, in0=pbe, in1=b2bb, op=OP.add)
            aeb = grp.tile([G, 2], f32, tag="aeb", name="aeb")
            nc.vector.tensor_tensor(out=aeb, in0=pae, in1=b2aeb, op=OP.add)

            # modulated (fp16); ing slices were pre-shifted in the prologue
            ing = ing_all[g]
            mod = grp.tile([G, D], f16, tag="mod", name="mod")
            nc.vector.tensor_tensor(out=mod, in0=ing, in1=gate16, op=OP.mult)
            nc.vector.tensor_tensor(out=mod, in0=mod, in1=beta16, op=OP.add)

            modT = grp.tile([128, 4, G], f16, tag="modT", name="modT")
            pTm = ps.tile([128, 4, G], f16, tag="grpP", bufs=1, name="pTm")
            for dc in range(4):
                pe_transpose_block(pTm[:, dc, :],
                                   mod[:, dc * 128:(dc + 1) * 128], ident16)
            nc.scalar.copy(out=modT, in_=pTm)
            fill(3)

            pk = ps.tile([G, D], f32, tag="grpP", bufs=1, name="pk")
            pv = ps.tile([G, D], f32, tag="grpP2", bufs=1, name="pv")
            pae2 = ps.tile([G, 2], f32, tag="vec", bufs=3, name="pae2")
            for dc in range(4):
                st, sp = (dc == 0), (dc == 3)
                nc.tensor.matmul(pk, lhsT=modT[:, dc, :],
                                 rhs=wkvT[:, dc, 0:D], start=st, stop=sp)
                nc.tensor.matmul(pv, lhsT=modT[:, dc, :],
                                 rhs=wkvT[:, dc, D:2 * D], start=st, stop=sp)
                nc.tensor.matmul(pae2, lhsT=modT[:, dc, :],
                                 rhs=wkvT[:, dc, 2 * D:TD], start=st, stop=sp)
            fill(4)

            kr = grp.tile([G, D], f32, tag="kr", name="kr")
            nc.scalar.copy(out=kr, in_=pk)
            kr16 = grp.tile([G, D], f16, tag="kr16", name="kr16")
            l2row(kr, "k", out=kr16)

            kT = grp.tile([128, 4, G], f16, tag="kT", bufs=2, name="kT")
            kTf = grp.tile([128, 4, G], f32, tag="kTf", bufs=2, name="kTf")
            pTk = ps.tile([128, 4, G], f16, tag="grpP", bufs=1, name="pTk")
            for jc in range(4):
                pe_transpose_block(pTk[:, jc, :],
                                   kr16[:, jc * 128:(jc + 1) * 128], ident16)
            nc.scalar.copy(out=kT, in_=pTk)
            nc.scalar.copy(out=kTf, in_=pTk)

            vv16 = grp.tile([G, D], f16, tag="vv16", bufs=2, name="vv16")
            nc.scalar.copy(out=vv16, in_=pv)

            nc.vector.tensor_tensor(out=aeb, in0=aeb, in1=pae2, op=OP.add)
            nc.scalar.activation(out=aeb, in_=aeb, func=AF.Sigmoid)
            # oma = 1 - alpha ; etn = -eta*scale (Act mul(perr, etn) = eta*err)
            oma = grp.tile([G, 1], f32, tag="oma", name="oma")
            nc.vector.tensor_scalar(oma, aeb[:, 0:1], -1.0, 1.0,
                                    op0=OP.mult, op1=OP.add)
            etn = grp.tile([G, 1], f32, tag="etn", name="etn")
            nc.vector.tensor_scalar(etn, aeb[:, 1:2], -(float(D) ** -0.5),
                                    None, op0=OP.mult)
            pomr = ps.tile([1, G], f32, tag="vec", bufs=3, name="pomr")
            pe_transpose_block(pomr, oma, ident)
            petar = ps.tile([1, G], f32, tag="vec", bufs=3, name="petar")
            pe_transpose_block(petar, etn, ident)
            omr16 = grp.tile([1, G], f16, tag="omr", name="omr")
            nc.scalar.copy(out=omr16, in_=pomr)
            etar = grp.tile([1, G], f32, tag="etar", bufs=2, name="etar")
            nc.scalar.copy(out=etar, in_=petar)
            pomb = ps.tile([128, G], f32, tag="vec", bufs=3, name="pomb")
            nc.tensor.matmul(pomb, lhsT=ones16, rhs=omr16, start=True,
                             stop=True)
            omab = grp.tile([128, G], f32, tag="omab", bufs=2, name="omab")
            nc.scalar.copy(out=omab, in_=pomb)

            fill(5)
            bss[g] = {"kT": kT, "kTf": kTf, "vv16": vv16, "omab": omab,
                      "etar": etar}

        # ---- merged schedule ----
        # step g emits: B(g) + C(g-1) + A(g+1).
        # Keep the PE p-state ramped through the group-0 load window with
        # dummy matmuls (output unused) so the first mc matvecs run warm.
        a_init(0)
        pwarm = ps.tile([128, D], f32, tag="bc2", bufs=2, name="pwarm")
        for _w in range(20):
            nc.tensor.matmul(pwarm, lhsT=ident16[:, 0:128],
                             rhs=w1T[:, 0, :], start=True, stop=True)
        a_emit(0, [x for ch in a_sched(0)[1] for x in []] or
               [x for ch in a_sched(0)[1:3] for x in ch])
        emit_inorm_ln()
        a_emit(0, [x for ch in a_sched(0)[3:] for x in ch])
        for g in range(NG):
            fills = [[] for _ in range(6)]
            if g - 1 >= 0:
                for i, ch in enumerate(c_sched(g - 1)):
                    fills[i].append(lambda g1=g - 1, c=ch: c_emit(g1, c))
            if g + 1 < NG:
                a_init(g + 1)
                for i, ch in enumerate(a_sched(g + 1)):
                    fills[i].append(lambda g1=g + 1, c=ch: a_emit(g1, c))
            stage_b(g, fills)
        for ch in c_sched(NG - 1):
            c_emit(NG - 1, ch, drain=True)
    nc.compile()
    return nc


def _prep_host(inputs):
    f = np.float32
    w_q = np.asarray(inputs["w_q"], f)
    w_k = np.asarray(inputs["w_k"], f)
    w_v = np.asarray(inputs["w_v"], f)
    w_a = np.asarray(inputs["w_alpha"], f).reshape(1, D)
    w_e = np.asarray(inputs["w_eta"], f).reshape(1, D)
    wkv = np.concatenate([w_k, w_v, w_a, w_e], axis=0)  # (1026, 512)
    b2 = np.asarray(inputs["mc_b2"], f)
    bae = np.stack([np.asarray(inputs["b_alpha"], f).reshape(()),
                    np.asarray(inputs["b_eta"], f).reshape(())])
    com = {
        "wqT": np.ascontiguousarray(w_q.T).astype(np.float16),
        "w1T": np.ascontiguousarray(np.asarray(inputs["mc_w1"], f).T).astype(
            np.float16),
        "w2T": np.ascontiguousarray(np.asarray(inputs["mc_w2"], f).T).astype(
            np.float16),
        "wkvT": np.ascontiguousarray(wkv.T).astype(np.float16),
        "npack": np.ascontiguousarray(np.concatenate([
            np.asarray(inputs["n1_g"], f), np.asarray(inputs["n1_b"], f)])
        ).astype(np.float16),
        "cpack": np.ascontiguousarray(np.concatenate([
            np.asarray(inputs["mc_ln_g"], f),
            np.asarray(inputs["mc_ln_b"], f),
            np.asarray(inputs["mc_b1"], f),
            b2[0:D], b2[D:2 * D], b2[2 * D:] + bae])).astype(np.float16),
    }
    old = np.asarray(inputs["old_state"], f)
    xs = np.asarray(inputs["user_static_emb"], f)
    it = np.asarray(inputs["item_emb"], f)
    in_maps = []
    for c in range(NCORES):
        s = slice(c * LB, (c + 1) * LB)
        m = dict(com)
        m["oldT"] = old[s].transpose(0, 2, 1).astype(np.float16)
        m["xs"] = np.ascontiguousarray(xs[s])
        m["it"] = np.ascontiguousarray(it[s])
        in_maps.append(m)
    return in_maps


def kernel(**inputs):
    from concourse import bass_utils

    if "nc" not in _CACHE:
        _CACHE["nc"] = _build()
    nc = _CACHE["nc"]
    in_maps = _prep_host(inputs)
    res = bass_utils.run_bass_kernel_spmd(nc, in_maps,
                                          core_ids=list(range(NCORES)))
    out = np.concatenate(
        [r["out"].transpose(0, 2, 1) for r in res.results], axis=0)
    return out.astype(np.float32)


if __name__ == "__main__":
    pass
